# revision 5
# baseline (speedup 1.0000x reference)
"""AGNNConv distributed Trainium2 kernel (8 NeuronCores, Bass/Tile).

reference math:
  X_prime = X @ W                                  [N, 64]
  att_e   = a * <xp[src_e], xp[dst_e]>             (a = attention_w[0,0])
  out[i]  = sum_{e: src_e = i} att_e * xp[dst_e]

Gram-form, per 128-node stripe, per 128-edge tile (edges bucketed by
(dst-chunk, src-stripe), any order within a bucket):
  A[e, n]  = <xp_dst_e, a*xp_n>     PE matmul over features (2 x K=32 halves)
  M[e, n]  = (src_e == base + n)    DVE iota + is_equal
  AM       = A * M (bf16)           DVE   (== att_e placed at column src_e)
  out_stripe[n, :] += matmul(lhsT=AM[e,n], rhs=dst_feat^T[e,f])   PE

Sharding: edges by src node range (edge_src sorted -> contiguous). Each core
computes its XpT shard (feature-on-partition, bf16, packed [32, npc, 2] with
features f/f+32 paired), AllGathers shards so each core holds the full table in
DRAM, then per dst-chunk phase gathers per-edge dst features from the
SBUF-resident chunk table with gpsimd.ap_gather (int16 indices). Output rows
are core-exclusive; host concatenates.
"""

import os
import sys
import numpy as np

for _p in ("/opt/trn_rl_repo", "/root/.axon_site/_ro/trn_rl_repo"):
    if os.path.isdir(_p) and _p not in sys.path:
        sys.path.append(_p)

from concourse import bacc, mybir, tile  # noqa: E402
from concourse.bass_utils import run_bass_kernel_spmd  # noqa: E402
from concourse.masks import make_identity  # noqa: E402

P = 128


class Cfg:
    def __init__(self, n_nodes=100000, in_dim=128, out_dim=64, cores=8,
                 chunk=25000, sgroup=4):
        self.n_nodes = n_nodes
        self.in_dim = in_dim
        self.out_dim = out_dim
        self.cores = cores
        self.npc = n_nodes // cores
        self.chunk = chunk
        self.nchunks = n_nodes // chunk
        self.nstripes = (self.npc + P - 1) // P
        self.sgroup = sgroup
        self.ngroups = (self.nstripes + sgroup - 1) // sgroup


CFG = Cfg()
_CACHE = {}


def _wrap16(idx, ch):
    n = idx.size
    w = idx.astype(np.int16).reshape(n // 16, 16).T
    return np.tile(w, (ch // 16, 1))


def _build_schedule(cfg, edge_src, edge_dst):
    src = np.asarray(edge_src).astype(np.int64)
    dst = np.asarray(edge_dst).astype(np.int64)
    bounds = np.searchsorted(src, np.arange(cfg.cores + 1) * cfg.npc)

    per_core = []
    cnt = np.zeros((cfg.cores, cfg.nchunks, cfg.nstripes), dtype=np.int64)
    for k in range(cfg.cores):
        s = src[bounds[k]:bounds[k + 1]] - k * cfg.npc
        d = dst[bounds[k]:bounds[k + 1]]
        c = d // cfg.chunk
        st = s // P
        order = np.lexsort((s, st, c))
        s, d, c, st = s[order], d[order], c[order], st[order]
        np.add.at(cnt[k], (c, st), 1)
        per_core.append((s, d, c, st))

    maxcnt = cnt.max(axis=0)
    tiles_cs = np.maximum(1, -(-maxcnt // P))
    slot_of = np.zeros((cfg.nchunks, cfg.nstripes), dtype=np.int64)
    tile_of = np.zeros((cfg.nchunks, cfg.nstripes), dtype=np.int64)
    group_slots = np.zeros((cfg.nchunks, cfg.ngroups), dtype=np.int64)
    slot = tl = 0
    for c in range(cfg.nchunks):
        for g in range(cfg.ngroups):
            g0 = slot
            for s in range(g * cfg.sgroup, min((g + 1) * cfg.sgroup, cfg.nstripes)):
                slot_of[c, s] = slot
                tile_of[c, s] = tl
                slot += int(tiles_cs[c, s]) * P
                tl += int(tiles_cs[c, s])
            group_slots[c, g] = slot - g0
    total_slots, total_tiles = slot, tl

    dsti = np.zeros((cfg.cores, total_slots), dtype=np.int16)
    srcc = np.full((cfg.cores, total_slots), 999.0, dtype=np.float32)
    for k in range(cfg.cores):
        s, d, c, st = per_core[k]
        if len(s) == 0:
            continue
        cell_key = c * cfg.nstripes + st
        change = np.r_[True, cell_key[1:] != cell_key[:-1]]
        starts = np.flatnonzero(change)
        idx = np.arange(len(s))
        pos = idx - np.repeat(idx[starts], np.diff(np.r_[starts, len(s)]))
        slots = slot_of[c, st] + pos
        dsti[k, slots] = (d - c * cfg.chunk).astype(np.int16)
        srcc[k, slots] = (s - st * P).astype(np.float32)

    srcc = srcc.reshape(cfg.cores, total_tiles, P).transpose(0, 2, 1).copy()
    dsti_w = np.stack([_wrap16(dsti[k], 32) for k in range(cfg.cores)])
    meta = dict(tiles_cs=tiles_cs, slot_of=slot_of, tile_of=tile_of,
                group_slots=group_slots, total_slots=total_slots,
                total_tiles=total_tiles, bounds=bounds)
    return meta, dsti_w, srcc


def build_graph(cfg, meta):
    bf16 = mybir.dt.bfloat16
    f32 = mybir.dt.float32
    tiles_cs = meta["tiles_cs"]
    tile_of = meta["tile_of"]
    slot_of = meta["slot_of"]
    group_slots = meta["group_slots"]
    total_slots = meta["total_slots"]
    total_tiles = meta["total_tiles"]
    NPC, F, IN = cfg.npc, cfg.out_dim, cfg.in_dim
    FH = F // 2
    XBLKS = (NPC + P - 1) // P
    SH_PER_CHUNK = cfg.chunk // NPC

    nc = bacc.Bacc()
    xs = nc.declare_dram_parameter("xs", [NPC, IN], f32, isOutput=False)
    w = nc.declare_dram_parameter("w", [IN, F], f32, isOutput=False)
    aw = nc.declare_dram_parameter("aw", [1, 1], f32, isOutput=False)
    dsti = nc.declare_dram_parameter("dsti", [32, total_slots // 16],
                                     mybir.dt.int16, isOutput=False)
    srcc = nc.declare_dram_parameter("srcc", [P, total_tiles], f32,
                                     isOutput=False)
    out = nc.declare_dram_parameter("out", [NPC, F], f32, isOutput=True)

    with tile.TileContext(nc) as tc:
        with (
            tc.tile_pool(name="const", bufs=1) as cp,
            tc.tile_pool(name="dram", bufs=1, space="DRAM") as dram,
        ):
            ident = cp.tile([P, P], f32)
            make_identity(nc, ident[:])
            ident32b = cp.tile([32, 32], bf16)
            nc.vector.tensor_copy(out=ident32b[:], in_=ident[:32, :32])
            iota = cp.tile([P, P], f32)
            iota_i = cp.tile([P, P], mybir.dt.int32)
            nc.gpsimd.iota(iota_i[:], pattern=[[1, P]], base=0,
                           channel_multiplier=0)
            nc.vector.tensor_copy(out=iota[:], in_=iota_i[:])

            w_sb = cp.tile([IN, F], f32)
            nc.sync.dma_start(w_sb[:], w[:, :])
            aw_sb = cp.tile([1, 1], f32)
            nc.sync.dma_start(aw_sb[:], aw[:, :])
            ones_row = cp.tile([1, 32], f32)
            nc.vector.memset(ones_row[:], 1.0)

            # a broadcast to [32, 1]
            NPAD = cfg.nstripes * P
            xpsA = cp.tile([32, NPAD], bf16)  # a-scaled XpT rows 0..32
            xpsB = cp.tile([32, NPAD], bf16)  # a-scaled XpT rows 32..64
            if NPAD > NPC:
                nc.vector.memset(xpsA[:, NPC:], 0.0)
                nc.vector.memset(xpsB[:, NPC:], 0.0)
            ag_in = dram.tile([32, NPC, 2], bf16)
            ag_out = dram.tile([cfg.cores, 32, NPC, 2], bf16)
            out_dram = dram.tile([cfg.nstripes, P, F], f32)

            # ---- Stage 1: XpT shard (packed + scaled halves) ----
            with (
                tc.tile_pool(name="xstage", bufs=3) as xp_pool,
                tc.tile_pool(name="xpsum", bufs=2, space="PSUM") as xps,
                tc.tile_pool(name="xpsum2", bufs=2, space="PSUM") as xps2,
                tc.tile_pool(name="shard", bufs=1) as shp,
            ):
                a_ps = xps.tile([32, 1], f32, tag="aps")
                nc.tensor.matmul(out=a_ps[:], lhsT=ones_row[:], rhs=aw_sb[:],
                                 start=True, stop=True)
                a_sb = cp.tile([32, 1], f32)
                nc.vector.tensor_copy(out=a_sb[:], in_=a_ps[:])

                tab_shard = shp.tile([32, NPC, 2], bf16)
                for b in range(XBLKS):
                    r0 = b * P
                    rows = min(P, NPC - r0)
                    xblk = xp_pool.tile([P, IN], f32, tag="xblk")
                    nc.sync.dma_start(xblk[:rows, :], xs[r0:r0 + rows, :])
                    xt_ps = xps.tile([IN, P], f32, tag="xt")
                    nc.tensor.matmul(out=xt_ps[:, :rows], lhsT=xblk[:rows, :],
                                     rhs=ident[:rows, :rows],
                                     start=True, stop=True)
                    xt_sb = xp_pool.tile([IN, P], f32, tag="xtsb")
                    nc.vector.tensor_copy(out=xt_sb[:, :rows],
                                          in_=xt_ps[:, :rows])
                    lo_ps = xps2.tile([32, P], f32, tag="lo")
                    hi_ps = xps2.tile([32, P], f32, tag="hi")
                    nc.tensor.matmul(out=lo_ps[:, :rows], lhsT=w_sb[:, 0:FH],
                                     rhs=xt_sb[:, :rows], start=True, stop=True)
                    nc.tensor.matmul(out=hi_ps[:, :rows], lhsT=w_sb[:, FH:F],
                                     rhs=xt_sb[:, :rows], start=True, stop=True)
                    nc.vector.tensor_copy(out=tab_shard[:, r0:r0 + rows, 0],
                                          in_=lo_ps[:, :rows])
                    nc.vector.tensor_copy(out=tab_shard[:, r0:r0 + rows, 1],
                                          in_=hi_ps[:, :rows])
                    nc.vector.tensor_scalar(
                        out=xpsA[:, r0:r0 + rows], in0=lo_ps[:, :rows],
                        scalar1=a_sb[:], scalar2=None,
                        op0=mybir.AluOpType.mult)
                    nc.vector.tensor_scalar(
                        out=xpsB[:, r0:r0 + rows], in0=hi_ps[:, :rows],
                        scalar1=a_sb[:], scalar2=None,
                        op0=mybir.AluOpType.mult)

                # ---- Stage 2: AllGather shards ----
                nc.sync.dma_start(ag_in[:], tab_shard[:])
                nc.gpsimd.collective_compute(
                    "AllGather", mybir.AluOpType.bypass,
                    replica_groups=[list(range(cfg.cores))],
                    ins=[ag_in.opt()], outs=[ag_out.opt()],
                )

            # ---- Stage 3: chunk phases ----
            with (
                tc.tile_pool(name="tabp", bufs=1) as tp,
                tc.tile_pool(name="gather", bufs=3) as gp,
                tc.tile_pool(name="work", bufs=4) as wp,
                tc.tile_pool(name="acc", bufs=3) as ap_,
                tc.tile_pool(name="psA", bufs=2, space="PSUM") as psA,
                tc.tile_pool(name="psT", bufs=2, space="PSUM") as psT,
                tc.tile_pool(name="psO", bufs=2, space="PSUM") as psO,
            ):
                for c in range(cfg.nchunks):
                    tab = tp.tile([32, cfg.chunk, 2], bf16, tag="tab")
                    for j in range(SH_PER_CHUNK):
                        nc.sync.dma_start(
                            tab[:, j * NPC:(j + 1) * NPC, :]
                            .rearrange("p a b -> p (a b)"),
                            ag_out[c * SH_PER_CHUNK + j]
                            .rearrange("p a b -> p (a b)"))
                    for g in range(cfg.ngroups):
                        nsl = int(group_slots[c, g])
                        if nsl == 0:
                            continue
                        s_lo = int(slot_of[c, g * cfg.sgroup])
                        t_lo = int(tile_of[c, g * cfg.sgroup])
                        git = gp.tile([32, nsl // 16], mybir.dt.int16, tag="git")
                        nc.sync.dma_start(
                            git[:], dsti[:, s_lo // 16:(s_lo + nsl) // 16])
                        dstT = gp.tile([32, nsl, 2], bf16, tag="dstT")
                        nc.gpsimd.ap_gather(
                            out_ap=dstT[:], in_ap=tab[:], idxs_ap=git[:],
                            channels=32, num_elems=cfg.chunk, d=2, num_idxs=nsl)
                        scg = wp.tile([P, nsl // P], f32, tag="scg")
                        nc.sync.dma_start(scg[:], srcc[:, t_lo:t_lo + nsl // P])

                        smax = min((g + 1) * cfg.sgroup, cfg.nstripes)
                        for s in range(g * cfg.sgroup, smax):
                            o_ps = psO.tile([P, F], f32, tag="ops")
                            ntile = int(tiles_cs[c, s])
                            for t in range(ntile):
                                col0 = int(slot_of[c, s]) - s_lo + t * P
                                d_lo = dstT[:, col0:col0 + P, 0]
                                d_hi = dstT[:, col0:col0 + P, 1]
                                a_ps2 = psA.tile([P, P], f32, tag="aps")
                                nc.tensor.matmul(
                                    out=a_ps2[:], lhsT=d_lo,
                                    rhs=xpsA[:, s * P:s * P + P],
                                    start=True, stop=False)
                                nc.tensor.matmul(
                                    out=a_ps2[:], lhsT=d_hi,
                                    rhs=xpsB[:, s * P:s * P + P],
                                    start=False, stop=True)
                                tA = psT.tile([P, FH], f32, tag="tA")
                                tB = psT.tile([P, FH], f32, tag="tB")
                                nc.tensor.matmul(out=tA[:], lhsT=d_lo,
                                                 rhs=ident32b[:],
                                                 start=True, stop=True)
                                nc.tensor.matmul(out=tB[:], lhsT=d_hi,
                                                 rhs=ident32b[:],
                                                 start=True, stop=True)
                                dtt = wp.tile([P, F], bf16, tag="dtt")
                                nc.vector.tensor_copy(out=dtt[:, 0:FH], in_=tA[:])
                                nc.vector.tensor_copy(out=dtt[:, FH:F], in_=tB[:])
                                gcol = int(tile_of[c, s]) - t_lo + t
                                m_t = wp.tile([P, P], f32, tag="mt")
                                nc.vector.tensor_scalar(
                                    out=m_t[:], in0=iota[:],
                                    scalar1=scg[:, gcol:gcol + 1], scalar2=None,
                                    op0=mybir.AluOpType.is_equal)
                                am = wp.tile([P, P], bf16, tag="am")
                                nc.vector.tensor_tensor(
                                    out=am[:], in0=a_ps2[:], in1=m_t[:],
                                    op=mybir.AluOpType.mult)
                                nc.tensor.matmul(out=o_ps[:], lhsT=am[:],
                                                 rhs=dtt[:], start=(t == 0),
                                                 stop=(t == ntile - 1))
                            if c == 0:
                                oacc = ap_.tile([P, F], f32, tag="oacc")
                                nc.vector.tensor_copy(out=oacc[:], in_=o_ps[:])
                                nc.sync.dma_start(out_dram[s], oacc[:])
                            else:
                                oacc = ap_.tile([P, F], f32, tag="oacc")
                                nc.sync.dma_start(oacc[:], out_dram[s])
                                nc.vector.tensor_add(oacc[:], oacc[:], o_ps[:])
                                if c == cfg.nchunks - 1:
                                    r0 = s * P
                                    rows = min(P, NPC - r0)
                                    nc.sync.dma_start(out[r0:r0 + rows, :],
                                                      oacc[:rows, :])
                                else:
                                    nc.sync.dma_start(out_dram[s], oacc[:])

    nc.finalize()
    return nc


def run(cfg, X, weights, attention_w, edge_src, edge_dst, trace=False):
    X = np.ascontiguousarray(np.asarray(X, dtype=np.float32))
    weights = np.ascontiguousarray(np.asarray(weights, dtype=np.float32))
    attention_w = np.ascontiguousarray(
        np.asarray(attention_w, dtype=np.float32)).reshape(1, 1)

    meta, dsti_w, srcc = _build_schedule(cfg, edge_src, edge_dst)
    key = (cfg.n_nodes, cfg.cores, meta["total_slots"], meta["total_tiles"],
           tuple(meta["tiles_cs"].ravel()))
    if key not in _CACHE:
        _CACHE.clear()
        _CACHE[key] = build_graph(cfg, meta)
    nc = _CACHE[key]

    in_maps = []
    for k in range(cfg.cores):
        in_maps.append({
            "xs": X[k * cfg.npc:(k + 1) * cfg.npc, :],
            "w": weights,
            "aw": attention_w,
            "dsti": dsti_w[k],
            "srcc": srcc[k],
        })
    res = run_bass_kernel_spmd(nc, in_maps, list(range(cfg.cores)),
                               trace=trace)
    run.last_exec_time_ns = res.exec_time_ns
    out = np.concatenate([res.results[k]["out"] for k in range(cfg.cores)],
                         axis=0)
    return out.astype(np.float32)


run.last_exec_time_ns = None


def _install_ntff_shim():
    """bass_utils under axon imports antenv.axon_hooks for NTFF profiling;
    this image lacks it — inject a stand-in built from trn_agent_boot."""
    import types
    if "antenv.axon_hooks" in sys.modules:
        return
    try:
        from trn_agent_boot.trn_boot import _ntff_profile_via_ctypes
        hook = _ntff_profile_via_ctypes("/opt/axon/libaxon_pjrt.so")
    except Exception:
        return
    mod = types.ModuleType("antenv.axon_hooks")
    mod.get_axon_ntff_profile_hook = lambda: hook
    mod.set_axon_ntff_profile_hook = lambda h: None
    sys.modules["antenv.axon_hooks"] = mod


def kernel(X, weights, attention_w, edge_src, edge_dst):
    trace = bool(os.environ.get("AGNN_TRACE"))
    if trace:
        _install_ntff_shim()
    out = run(CFG, X, weights, attention_w, edge_src, edge_dst, trace=trace)
    kernel.last_exec_time_ns = run.last_exec_time_ns
    return out


kernel.last_exec_time_ns = None
